# revision 1
# baseline (speedup 1.0000x reference)
"""Distributed Trainium2 kernel for single-head causal attention.

Problem: B=4, S=2048, d_model=d_attn=1024, f32 I/O.
  q = x@Wq.T; k = x@Wk.T; v = x@Wv.T
  logits = q@k.T  (causal + padding mask, then /sqrt(d_model))
  out = softmax(logits)@v @ Wo.T + bo

Sharding (8 cores, no collectives): core = (batch b, group g).
  g=0 owns query rows {0:512, 1536:2048}, g=1 owns {512:1024, 1024:1536}
  -> causal work is balanced (both groups cover 20 useful k-tiles).
  K/V are computed redundantly by both cores of a batch (cheaper than a
  2-rank all-gather at ~35 GB/s).

Everything is laid out transposed host-side (xt=[d,s], W^T=[d,a], Wo^T=[a,m],
all bf16) so the on-device chain
  QT[a,q] -> KT[a,k] -> V[k,a] -> ST[k,q] -> PT[k,q] -> OT[a,q] -> YT[m,q]
needs no on-device transposes. Causality is applied as host-computed
multiplicative 0/1 masks on PT (post-exp); softmax denominators come from
ones-vector matmuls ([1,q] row), normalization is fused into the OT
PSUM->SBUF copy via a partition-broadcast reciprocal row.

The SPMD graph is uniform across cores: q-block 0 runs 8 k-tiles, q-block 1
runs 16; per-core masks zero out the over-computed tiles.
"""

import os
import sys

sys.path.insert(0, "/opt/trn_rl_repo")

import numpy as np
import ml_dtypes

import concourse.bass as bass  # noqa: F401  (engine types)
import concourse.mybir as mybir
from concourse.bacc import Bacc
from concourse.tile import TileContext
from concourse.bass_utils import run_bass_kernel_spmd

BF = mybir.dt.bfloat16
F32 = mybir.dt.float32
BF_NP = ml_dtypes.bfloat16

P = 128          # partitions / tile edge
B, S, D = 4, 2048, 1024
DC = D // P      # 8 chunks of the contraction (d / a) axis
QW = 512         # q-block width (matmul moving free dim)
NQB = 2          # q-blocks per core (each QW wide -> 1024 q rows/core)
NKT = (8, 16)    # k-tiles per q-block (uniform SPMD profile)
KT_TOT = 16      # total k-tiles (S / P)
SCALE = 1.0 / 32.0  # 1/sqrt(d_model)

# global q-row starts of each q-block, per group
QSTARTS = ((0, 3 * QW), (QW, 2 * QW))

_NC_CACHE = None
LAST_RESULT = None  # BassKernelResults of the last run (for test.py)


def _build():
    nc = Bacc("TRN2")
    xt = nc.dram_tensor("xt", [D, S], BF, kind="ExternalInput")
    xtq = nc.dram_tensor("xtq", [D, NQB * QW], BF, kind="ExternalInput")
    wqt = nc.dram_tensor("wqt", [D, D], BF, kind="ExternalInput")
    wkt = nc.dram_tensor("wkt", [D, D], BF, kind="ExternalInput")
    wvt = nc.dram_tensor("wvt", [D, D], BF, kind="ExternalInput")
    wot = nc.dram_tensor("wot", [D, D], BF, kind="ExternalInput")
    bo_d = nc.dram_tensor("bo", [DC, P, 1], F32, kind="ExternalInput")
    masks = nc.dram_tensor("masks", [KT_TOT, P, QW], BF, kind="ExternalInput")
    out = nc.dram_tensor("out", [D, NQB * QW], F32, kind="ExternalOutput")

    with TileContext(nc) as tc:
        with (
            tc.tile_pool(name="persist", bufs=1) as pp,
            tc.tile_pool(name="psum", bufs=1, space="PSUM") as ps,
        ):
            # resident activations
            qt_s = pp.tile([P, DC, NQB * QW], BF, tag="qt")   # QT [a, q]
            kt_s = pp.tile([P, DC, S], BF, tag="kt")          # KT [a, k]
            v_s = pp.tile([P, KT_TOT, D], BF, tag="v")        # V  [k, a]
            wo_s = pp.tile([P, DC, D], BF, tag="wo")          # Wo^T [a, m]
            bo_s = pp.tile([P, DC, 1], F32, tag="bo")
            ones_s = pp.tile([P, 1], BF, tag="ones")
            nc.vector.memset(ones_s[:], 1.0)
            nc.sync.dma_start(wo_s[:], wot.rearrange("(c p) m -> p c m", p=P))
            nc.sync.dma_start(bo_s[:], bo_d.rearrange("c p o -> p c o"))

            with tc.tile_pool(name="xw", bufs=1) as xw:
                xts, xtqs, wq_c, wk_c, wv_c = [], [], [], [], []
                for c in range(DC):
                    t = xw.tile([P, S], BF, tag=f"xt{c}")
                    nc.sync.dma_start(t[:], xt[c * P:(c + 1) * P, :])
                    xts.append(t)
                for c in range(DC):
                    t = xw.tile([P, NQB * QW], BF, tag=f"xq{c}")
                    nc.sync.dma_start(t[:], xtq[c * P:(c + 1) * P, :])
                    xtqs.append(t)
                for c in range(DC):
                    t = xw.tile([P, D], BF, tag=f"wq{c}")
                    nc.sync.dma_start(t[:], wqt[c * P:(c + 1) * P, :])
                    wq_c.append(t)
                for c in range(DC):
                    t = xw.tile([P, D], BF, tag=f"wk{c}")
                    nc.sync.dma_start(t[:], wkt[c * P:(c + 1) * P, :])
                    wk_c.append(t)
                for c in range(DC):
                    t = xw.tile([P, D], BF, tag=f"wv{c}")
                    nc.sync.dma_start(t[:], wvt[c * P:(c + 1) * P, :])
                    wv_c.append(t)

                # ---- projections (contraction over d, 8 chunks) ----
                # QT[a,q]: lhsT = wqT[d, a-tile], rhs = xtq[d, q-block]
                for at in range(DC):
                    for qb in range(NQB):
                        acc = ps.tile([P, QW], F32, tag="pp", bufs=2)
                        for d in range(DC):
                            nc.tensor.matmul(
                                acc[:],
                                wq_c[d][:, at * P:(at + 1) * P],
                                xtqs[d][:, qb * QW:(qb + 1) * QW],
                                start=(d == 0), stop=(d == DC - 1),
                            )
                        nc.vector.tensor_copy(
                            qt_s[:, at, qb * QW:(qb + 1) * QW], acc[:]
                        )
                # KT[a,k]: rhs = xt[d, k-block] (4 blocks of 512)
                for at in range(DC):
                    for kb in range(S // QW):
                        acc = ps.tile([P, QW], F32, tag="pp", bufs=2)
                        for d in range(DC):
                            nc.tensor.matmul(
                                acc[:],
                                wk_c[d][:, at * P:(at + 1) * P],
                                xts[d][:, kb * QW:(kb + 1) * QW],
                                start=(d == 0), stop=(d == DC - 1),
                            )
                        nc.vector.tensor_copy(
                            kt_s[:, at, kb * QW:(kb + 1) * QW], acc[:]
                        )
                # V[k,a]: lhsT = xt[d, k-tile], rhs = wvT[d, a-block]
                for kt in range(KT_TOT):
                    for ab in range(D // QW):
                        acc = ps.tile([P, QW], F32, tag="pp", bufs=2)
                        for d in range(DC):
                            nc.tensor.matmul(
                                acc[:],
                                xts[d][:, kt * P:(kt + 1) * P],
                                wv_c[d][:, ab * QW:(ab + 1) * QW],
                                start=(d == 0), stop=(d == DC - 1),
                            )
                        nc.vector.tensor_copy(
                            v_s[:, kt, ab * QW:(ab + 1) * QW], acc[:]
                        )

            # ---- attention (per q-block) ----
            with tc.tile_pool(name="attn", bufs=1) as ap:
                mask_s = ap.tile([P, KT_TOT, QW], BF, tag="mask")
                nc.sync.dma_start(mask_s[:], masks.rearrange("t p j -> p t j"))

                for qb in range(NQB):
                    nkt = NKT[qb]
                    kts = list(range(nkt))  # k-tiles used by this q-block
                    q_sl = slice(qb * QW, (qb + 1) * QW)

                    # ST[k,q] = KT.T @ QT, exp, mask -> PT (bf16, SBUF)
                    pts = []
                    for kt in kts:
                        st = ps.tile([P, QW], F32, tag="st", bufs=2)
                        for ac in range(DC):
                            nc.tensor.matmul(
                                st[:],
                                kt_s[:, ac, kt * P:(kt + 1) * P],
                                qt_s[:, ac, q_sl],
                                start=(ac == 0), stop=(ac == DC - 1),
                            )
                        pt = ap.tile([P, QW], BF, tag=f"pt{qb}_{kt}")
                        nc.scalar.activation(
                            pt[:], st[:], mybir.ActivationFunctionType.Exp,
                            scale=SCALE,
                        )
                        # qb0: every k-tile is masked; qb1: only tiles 8..15
                        if qb == 0 or kt >= 8:
                            nc.vector.tensor_mul(pt[:], pt[:], mask_s[:, kt, :])
                        pts.append(pt)

                    # colsum[1,q] = sum_k PT  (ones-vector matmuls)
                    cs = ps.tile([1, QW], F32, tag="cs", bufs=1)
                    for i, kt in enumerate(kts):
                        nc.tensor.matmul(
                            cs[:], ones_s[:], pts[i][:],
                            start=(i == 0), stop=(i == len(kts) - 1),
                        )
                    recip = ap.tile([1, QW], F32, tag="recip", bufs=2)
                    nc.vector.reciprocal(recip[:], cs[:])
                    rb = ap.tile([P, QW], F32, tag="rb", bufs=2)
                    nc.gpsimd.partition_broadcast(rb[:], recip[:])

                    # OT[a,q] = V.T @ PT (accumulate over k), normalize on copy
                    ots = []
                    for at in range(DC):
                        ot = ps.tile([P, QW], F32, tag="ot", bufs=2)
                        for i, kt in enumerate(kts):
                            nc.tensor.matmul(
                                ot[:],
                                v_s[:, kt, at * P:(at + 1) * P],
                                pts[i][:],
                                start=(i == 0), stop=(i == len(kts) - 1),
                            )
                        ot_sb = ap.tile([P, QW], BF, tag=f"ot{at}", bufs=2)
                        nc.vector.tensor_mul(ot_sb[:], ot[:], rb[:])
                        ots.append(ot_sb)

                    # YT[m,q] = WoT.T @ OT (accumulate over a) + bo
                    for mt in range(DC):
                        yt = ps.tile([P, QW], F32, tag="yt", bufs=1)
                        for ac in range(DC):
                            nc.tensor.matmul(
                                yt[:],
                                wo_s[:, ac, mt * P:(mt + 1) * P],
                                ots[ac][:],
                                start=(ac == 0), stop=(ac == DC - 1),
                            )
                        yt_sb = ap.tile([P, QW], F32, tag="yt_sb", bufs=2)
                        nc.vector.tensor_scalar_add(yt_sb[:], yt[:], bo_s[:, mt, :])
                        nc.sync.dma_start(
                            out[mt * P:(mt + 1) * P, q_sl], yt_sb[:]
                        )

    nc.compile()
    return nc


def _get_nc():
    global _NC_CACHE
    if _NC_CACHE is None:
        _NC_CACHE = _build()
    return _NC_CACHE


def kernel(x, mask, Wq, Wk, Wv, Wo, bo):
    global LAST_RESULT
    x = np.asarray(x, dtype=np.float32)
    mask = np.asarray(mask, dtype=np.float32)
    Wq = np.asarray(Wq, dtype=np.float32)
    Wk = np.asarray(Wk, dtype=np.float32)
    Wv = np.asarray(Wv, dtype=np.float32)
    Wo = np.asarray(Wo, dtype=np.float32)
    bo = np.asarray(bo, dtype=np.float32)

    wqt = Wq.T.astype(BF_NP).copy()
    wkt = Wk.T.astype(BF_NP).copy()
    wvt = Wv.T.astype(BF_NP).copy()
    wot = Wo.T.astype(BF_NP).copy()
    bo_r = np.ascontiguousarray(bo.reshape(DC, P, 1))

    in_maps = []
    for c in range(8):
        b, g = divmod(c, 2)
        xt = x[b].T.astype(BF_NP).copy()                       # [d, s]
        qcols = np.r_[QSTARTS[g][0]:QSTARTS[g][0] + QW,
                      QSTARTS[g][1]:QSTARTS[g][1] + QW]
        xtq = np.ascontiguousarray(xt[:, qcols])               # [d, 1024]

        m = np.zeros((KT_TOT, P, QW), dtype=np.float32)
        ki = np.arange(P)[:, None]
        qi = np.arange(QW)[None, :]
        for qb in range(NQB):
            q0 = QSTARTS[g][qb]
            for slot in range(8):
                kt = slot if qb == 0 else 8 + slot
                k0 = kt * P
                mm = ((k0 + ki) <= (q0 + qi)).astype(np.float32)
                mm *= mask[b, k0:k0 + P, None]                 # key padding
                m[kt] = mm
        in_maps.append({
            "xt": xt,
            "xtq": xtq,
            "wqt": wqt,
            "wkt": wkt,
            "wvt": wvt,
            "wot": wot,
            "bo": bo_r,
            "masks": m.astype(BF_NP),
        })

    nc = _get_nc()
    res = run_bass_kernel_spmd(
        nc, in_maps, core_ids=list(range(8)),
        trace=bool(os.environ.get("ATTN_TRACE")),
    )
    LAST_RESULT = res

    outp = np.empty((B, S, D), dtype=np.float32)
    for c in range(8):
        b, g = divmod(c, 2)
        yt = res.results[c]["out"]                             # [m, q_local]
        for qb in range(NQB):
            q0 = QSTARTS[g][qb]
            outp[b, q0:q0 + QW, :] = yt[:, qb * QW:(qb + 1) * QW].T
    return outp


# revision 9
# speedup vs baseline: 1.0798x; 1.0798x over previous
"""Distributed Trainium2 kernel for single-head causal attention.

Problem: B=4, S=2048, d_model=d_attn=1024, f32 I/O.
  q = x@Wq.T; k = x@Wk.T; v = x@Wv.T
  logits = q@k.T  (causal + padding mask, then /sqrt(d_model))
  out = softmax(logits)@v @ Wo.T + bo

Sharding (8 cores, no collectives): core = (batch b, group g).
  g=0 owns query rows {0:512, 1536:2048}, g=1 owns {512:1024, 1024:1536}
  -> causal work is balanced (both groups cover 20 useful k-tiles).
  K/V are computed redundantly by both cores of a batch (cheaper than a
  2-rank all-gather at ~35 GB/s).

Everything is laid out transposed host-side (xt=[d,s], W^T=[d,a], Wo^T=[a,m],
all bf16) so the on-device chain
  QT[a,q] -> KT[a,k] -> V[k,a] -> ST[k,q] -> PT[k,q] -> OT[a,q] -> YT[m,q]
needs no on-device transposes. Causality is applied as host-computed
multiplicative 0/1 masks on PT (post-exp); softmax denominators come from
ones-vector matmuls ([1,q] row), normalization is fused into the OT
PSUM->SBUF copy via a partition-broadcast reciprocal row.

The SPMD graph is uniform across cores: q-block 0 runs 8 k-tiles, q-block 1
runs 16; per-core masks zero out the over-computed tiles.
"""

import os
import sys

sys.path.insert(0, "/opt/trn_rl_repo")

import numpy as np
import ml_dtypes

import concourse.bass as bass  # noqa: F401  (engine types)
import concourse.mybir as mybir
from concourse.bacc import Bacc
from concourse.tile import TileContext
from concourse.bass_utils import run_bass_kernel_spmd

BF = mybir.dt.bfloat16
F32 = mybir.dt.float32
BF_NP = ml_dtypes.bfloat16

P = 128          # partitions / tile edge
B, S, D = 4, 2048, 1024
DC = D // P      # 8 chunks of the contraction (d / a) axis
QW = 512         # q-block width (matmul moving free dim)
NQB = 2          # q-blocks per core (each QW wide -> 1024 q rows/core)
NKT = (8, 16)    # k-tiles per q-block (uniform SPMD profile)
KT_TOT = 16      # total k-tiles (S / P)
SCALE = 1.0 / 32.0  # 1/sqrt(d_model)

# global q-row starts of each q-block, per group
QSTARTS = ((0, 3 * QW), (QW, 2 * QW))

_NC_CACHE = None
LAST_RESULT = None  # BassKernelResults of the last run (for test.py)


def _build():
    nc = Bacc("TRN2")
    xt = nc.dram_tensor("xt", [D, S], BF, kind="ExternalInput")
    xtq = nc.dram_tensor("xtq", [D, NQB * QW], BF, kind="ExternalInput")
    wqt = nc.dram_tensor("wqt", [D, D], BF, kind="ExternalInput")
    wkt = nc.dram_tensor("wkt", [D, D], BF, kind="ExternalInput")
    wvt = nc.dram_tensor("wvt", [D, D], BF, kind="ExternalInput")
    wot = nc.dram_tensor("wot", [D, D], BF, kind="ExternalInput")
    bo_d = nc.dram_tensor("bo", [DC, P, 1], F32, kind="ExternalInput")
    masks = nc.dram_tensor("masks", [KT_TOT, P, QW], BF, kind="ExternalInput")
    out = nc.dram_tensor("out", [D, NQB * QW], F32, kind="ExternalOutput")

    with TileContext(nc) as tc:
        with tc.tile_pool(name="persist", bufs=1) as pp:
            # resident activations
            qt_s = pp.tile([P, DC, NQB * QW], BF, tag="qt")   # QT [a, q]
            kt_s = pp.tile([P, DC, S], BF, tag="kt")          # KT [a, k]
            v_s = pp.tile([P, KT_TOT, D], BF, tag="v")        # V  [k, a]
            wo_s = pp.tile([P, DC, D], BF, tag="wo")          # Wo^T [a, m]
            bo_s = pp.tile([P, DC, 1], F32, tag="bo")
            ones_s = pp.tile([P, 1], BF, tag="ones")
            ones_r = pp.tile([1, P], mybir.dt.float32r, tag="ones_r")  # rank-1 bcast lhsT
            ones_rf = pp.tile([1, P], F32, tag="ones_rf")
            nc.vector.memset(ones_s[:], 1.0)
            nc.vector.memset(ones_rf[:], 1.0)
            with nc.allow_low_precision(reason="1.0 is exact in fp22"):
                nc.vector.tensor_copy(ones_r[:], ones_rf[:])

            with tc.tile_pool(name="xw", bufs=1) as xw:
                # DMA issue order = consumption order: QT operands first so
                # the PE starts ~2us in, then KT's, then V's, then epilogue.
                xts, xtqs, wq_c, wk_c, wv_c = [], [], [], [], []
                for c in range(DC):
                    t = xw.tile([P, NQB * QW], BF, tag=f"xq{c}")
                    nc.sync.dma_start(t[:], xtq[c * P:(c + 1) * P, :])
                    xtqs.append(t)
                    t = xw.tile([P, D], BF, tag=f"wq{c}")
                    nc.sync.dma_start(t[:], wqt[c * P:(c + 1) * P, :])
                    wq_c.append(t)
                for c in range(DC):
                    t = xw.tile([P, S], BF, tag=f"xt{c}")
                    nc.sync.dma_start(t[:], xt[c * P:(c + 1) * P, :])
                    xts.append(t)
                for c in range(DC):
                    t = xw.tile([P, D], BF, tag=f"wk{c}")
                    nc.sync.dma_start(t[:], wkt[c * P:(c + 1) * P, :])
                    wk_c.append(t)
                for c in range(DC):
                    t = xw.tile([P, D], BF, tag=f"wv{c}")
                    nc.sync.dma_start(t[:], wvt[c * P:(c + 1) * P, :])
                    wv_c.append(t)
                nc.sync.dma_start(wo_s[:], wot.rearrange("(c p) m -> p c m", p=P))
                nc.sync.dma_start(bo_s[:], bo_d.rearrange("c p o -> p c o"))

                ps = tc.alloc_tile_pool(name="proj_psum", bufs=1, space="PSUM")

                # ---- projections (contraction over d, 8 chunks) ----
                # QT[a,q]: lhsT = wqT[d, a-tile], rhs = xtq[d, q-block]
                for at in range(DC):
                    for qb in range(NQB):
                        acc = ps.tile([P, QW], F32, tag="pp", bufs=2)
                        for d in range(DC):
                            nc.tensor.matmul(
                                acc[:],
                                wq_c[d][:, at * P:(at + 1) * P],
                                xtqs[d][:, qb * QW:(qb + 1) * QW],
                                start=(d == 0), stop=(d == DC - 1),
                            )
                        nc.vector.tensor_copy(
                            qt_s[:, at, qb * QW:(qb + 1) * QW], acc[:]
                        )
                # KT[a,k]: rhs = xt[d, k-block] (4 blocks of 512)
                for at in range(DC):
                    for kb in range(S // QW):
                        acc = ps.tile([P, QW], F32, tag="pp", bufs=2)
                        for d in range(DC):
                            nc.tensor.matmul(
                                acc[:],
                                wk_c[d][:, at * P:(at + 1) * P],
                                xts[d][:, kb * QW:(kb + 1) * QW],
                                start=(d == 0), stop=(d == DC - 1),
                            )
                        nc.vector.tensor_copy(
                            kt_s[:, at, kb * QW:(kb + 1) * QW], acc[:]
                        )
                # V[k,a]: lhsT = xt[d, k-tile], rhs = wvT[d, a-block]
                for kt in range(KT_TOT):
                    for ab in range(D // QW):
                        acc = ps.tile([P, QW], F32, tag="pp", bufs=2)
                        for d in range(DC):
                            nc.tensor.matmul(
                                acc[:],
                                xts[d][:, kt * P:(kt + 1) * P],
                                wv_c[d][:, ab * QW:(ab + 1) * QW],
                                start=(d == 0), stop=(d == DC - 1),
                            )
                        nc.vector.tensor_copy(
                            v_s[:, kt, ab * QW:(ab + 1) * QW], acc[:]
                        )
                ps.release()

            # ---- attention (per q-block) ----
            with (
                tc.tile_pool(name="attn", bufs=1) as ap,
                tc.tile_pool(name="attn_psum", bufs=1, space="PSUM") as ps,
            ):
                mask_s = ap.tile([P, KT_TOT, QW], BF, tag="mask")
                nc.sync.dma_start(mask_s[:], masks.rearrange("t p j -> p t j"))

                for qb in range(NQB):
                    nkt = NKT[qb]
                    kts = list(range(nkt))  # k-tiles used by this q-block
                    q_sl = slice(qb * QW, (qb + 1) * QW)

                    # ST[k,q] = KT.T @ QT, exp, mask -> PT (bf16, SBUF)
                    pts = []
                    for kt in kts:
                        st = ps.tile([P, QW], F32, tag="st", bufs=2)
                        for ac in range(DC):
                            nc.tensor.matmul(
                                st[:],
                                kt_s[:, ac, kt * P:(kt + 1) * P],
                                qt_s[:, ac, q_sl],
                                start=(ac == 0), stop=(ac == DC - 1),
                            )
                        pt = ap.tile([P, QW], BF, tag=f"pt{qb}_{kt}")
                        nc.scalar.activation(
                            pt[:], st[:], mybir.ActivationFunctionType.Exp,
                            scale=SCALE,
                        )
                        # qb0: every k-tile is masked; qb1: only tiles 8..15
                        if qb == 0 or kt >= 8:
                            nc.vector.tensor_mul(pt[:], pt[:], mask_s[:, kt, :])
                        pts.append(pt)

                    # colsum[1,q] = sum_k PT  (ones-vector matmuls)
                    cs = ps.tile([1, QW], F32, tag="cs", bufs=1)
                    for i, kt in enumerate(kts):
                        nc.tensor.matmul(
                            cs[:], ones_s[:], pts[i][:],
                            start=(i == 0), stop=(i == len(kts) - 1),
                        )
                    recip = ap.tile([1, QW], mybir.dt.float32r, tag="recip", bufs=2)
                    with nc.allow_low_precision(
                        reason="f32r (fp22) reciprocal row: 6e-5 rel err, "
                        "far below the bf16 noise floor of this kernel"
                    ):
                        nc.vector.reciprocal(recip[:], cs[:])
                    # broadcast [1,q] -> [128,q] as a rank-1 PE matmul
                    # (keeps GpSimd off the critical path), then ACT-copy
                    # PSUM -> SBUF for the DVE normalize reads.
                    rb_ps = ps.tile([P, QW], F32, tag="rb", bufs=1)
                    nc.tensor.matmul(
                        rb_ps[:],
                        ones_r[:],
                        recip[:],
                        start=True, stop=True,
                    )
                    rb = ap.tile([P, QW], F32, tag="rb_sb", bufs=2)
                    nc.scalar.activation(
                        rb[:], rb_ps[:], mybir.ActivationFunctionType.Copy
                    )

                    # OT[a,q] = V.T @ PT (accumulate over k), normalize on copy
                    ots = []
                    for at in range(DC):
                        ot = ps.tile([P, QW], F32, tag="ot", bufs=2)
                        for i, kt in enumerate(kts):
                            nc.tensor.matmul(
                                ot[:],
                                v_s[:, kt, at * P:(at + 1) * P],
                                pts[i][:],
                                start=(i == 0), stop=(i == len(kts) - 1),
                            )
                        ot_sb = ap.tile([P, QW], BF, tag=f"ot{at}", bufs=2)
                        nc.vector.tensor_mul(ot_sb[:], ot[:], rb[:])
                        ots.append(ot_sb)

                    # YT[m,q] = WoT.T @ OT (accumulate over a) + bo
                    for mt in range(DC):
                        yt = ps.tile([P, QW], F32, tag="yt", bufs=2)
                        for ac in range(DC):
                            nc.tensor.matmul(
                                yt[:],
                                wo_s[:, ac, mt * P:(mt + 1) * P],
                                ots[ac][:],
                                start=(ac == 0), stop=(ac == DC - 1),
                            )
                        yt_sb = ap.tile([P, QW], F32, tag="yt_sb", bufs=2)
                        nc.vector.tensor_scalar_add(yt_sb[:], yt[:], bo_s[:, mt, :])
                        nc.sync.dma_start(
                            out[mt * P:(mt + 1) * P, q_sl], yt_sb[:]
                        )

    nc.compile()
    return nc


def _get_nc():
    global _NC_CACHE
    if _NC_CACHE is None:
        _NC_CACHE = _build()
    return _NC_CACHE


def kernel(x, mask, Wq, Wk, Wv, Wo, bo):
    global LAST_RESULT
    x = np.asarray(x, dtype=np.float32)
    mask = np.asarray(mask, dtype=np.float32)
    Wq = np.asarray(Wq, dtype=np.float32)
    Wk = np.asarray(Wk, dtype=np.float32)
    Wv = np.asarray(Wv, dtype=np.float32)
    Wo = np.asarray(Wo, dtype=np.float32)
    bo = np.asarray(bo, dtype=np.float32)

    wqt = Wq.T.astype(BF_NP).copy()
    wkt = Wk.T.astype(BF_NP).copy()
    wvt = Wv.T.astype(BF_NP).copy()
    wot = Wo.T.astype(BF_NP).copy()
    bo_r = np.ascontiguousarray(bo.reshape(DC, P, 1))

    in_maps = []
    for c in range(8):
        b, g = divmod(c, 2)
        xt = x[b].T.astype(BF_NP).copy()                       # [d, s]
        qcols = np.r_[QSTARTS[g][0]:QSTARTS[g][0] + QW,
                      QSTARTS[g][1]:QSTARTS[g][1] + QW]
        xtq = np.ascontiguousarray(xt[:, qcols])               # [d, 1024]

        m = np.zeros((KT_TOT, P, QW), dtype=np.float32)
        ki = np.arange(P)[:, None]
        qi = np.arange(QW)[None, :]
        for qb in range(NQB):
            q0 = QSTARTS[g][qb]
            for slot in range(8):
                kt = slot if qb == 0 else 8 + slot
                k0 = kt * P
                mm = ((k0 + ki) <= (q0 + qi)).astype(np.float32)
                mm *= mask[b, k0:k0 + P, None]                 # key padding
                m[kt] = mm
        in_maps.append({
            "xt": xt,
            "xtq": xtq,
            "wqt": wqt,
            "wkt": wkt,
            "wvt": wvt,
            "wot": wot,
            "bo": bo_r,
            "masks": m.astype(BF_NP),
        })

    nc = _get_nc()
    res = run_bass_kernel_spmd(
        nc, in_maps, core_ids=list(range(8)),
        trace=bool(os.environ.get("ATTN_TRACE")),
    )
    LAST_RESULT = res

    outp = np.empty((B, S, D), dtype=np.float32)
    for c in range(8):
        b, g = divmod(c, 2)
        yt = res.results[c]["out"]                             # [m, q_local]
        for qb in range(NQB):
            q0 = QSTARTS[g][qb]
            outp[b, q0:q0 + QW, :] = yt[:, qb * QW:(qb + 1) * QW].T
    return outp


# revision 13
# speedup vs baseline: 1.0885x; 1.0080x over previous
"""Distributed Trainium2 kernel for single-head causal attention.

Problem: B=4, S=2048, d_model=d_attn=1024, f32 I/O.
  q = x@Wq.T; k = x@Wk.T; v = x@Wv.T
  logits = q@k.T  (causal + padding mask, then /sqrt(d_model))
  out = softmax(logits)@v @ Wo.T + bo

Sharding (8 cores, no collectives): core = (batch b, group g).
  g=0 owns query rows {0:512, 1536:2048}, g=1 owns {512:1024, 1024:1536}
  -> causal work is balanced (both groups cover 20 useful k-tiles).
  K/V are computed redundantly by both cores of a batch (cheaper than a
  2-rank all-gather at ~35 GB/s).

Everything is laid out transposed host-side (xt=[d,s], W^T=[d,a], Wo^T=[a,m],
all bf16) so the on-device chain
  QT[a,q] -> KT[a,k] -> V[k,a] -> ST[k,q] -> PT[k,q] -> OT[a,q] -> YT[m,q]
needs no on-device transposes. Causality is applied as host-computed
multiplicative 0/1 masks on PT (post-exp); softmax denominators come from
ones-vector matmuls ([1,q] row), normalization is fused into the OT
PSUM->SBUF copy via a partition-broadcast reciprocal row.

The SPMD graph is uniform across cores: q-block 0 runs 8 k-tiles, q-block 1
runs 16; per-core masks zero out the over-computed tiles.
"""

import os
import sys

sys.path.insert(0, "/opt/trn_rl_repo")

import numpy as np
import ml_dtypes

import concourse.bass as bass  # noqa: F401  (engine types)
import concourse.mybir as mybir
from concourse.bacc import Bacc
from concourse.tile import TileContext
from concourse.bass_utils import run_bass_kernel_spmd

BF = mybir.dt.bfloat16
F32 = mybir.dt.float32
BF_NP = ml_dtypes.bfloat16

P = 128          # partitions / tile edge
B, S, D = 4, 2048, 1024
DC = D // P      # 8 chunks of the contraction (d / a) axis
QW = 512         # q-block width (matmul moving free dim)
NQB = 2          # q-blocks per core (each QW wide -> 1024 q rows/core)
NKT = (8, 16)    # k-tiles per q-block (uniform SPMD profile)
KT_TOT = 16      # total k-tiles (S / P)
SCALE = 1.0 / 32.0  # 1/sqrt(d_model)

# global q-row starts of each q-block, per group
QSTARTS = ((0, 3 * QW), (QW, 2 * QW))

_NC_CACHE = None
LAST_RESULT = None  # BassKernelResults of the last run (for test.py)


def _build():
    nc = Bacc("TRN2")
    xt = nc.dram_tensor("xt", [D, S], BF, kind="ExternalInput")
    xtq = nc.dram_tensor("xtq", [D, NQB * QW], BF, kind="ExternalInput")
    wqt = nc.dram_tensor("wqt", [D, D], BF, kind="ExternalInput")
    wkt = nc.dram_tensor("wkt", [D, D], BF, kind="ExternalInput")
    wvt = nc.dram_tensor("wvt", [D, D], BF, kind="ExternalInput")
    wot = nc.dram_tensor("wot", [D, D], BF, kind="ExternalInput")
    bo_d = nc.dram_tensor("bo", [DC, P, 1], F32, kind="ExternalInput")
    masks = nc.dram_tensor("masks", [KT_TOT, P, QW], BF, kind="ExternalInput")
    out = nc.dram_tensor("out", [NQB, D, QW], BF, kind="ExternalOutput")

    with TileContext(nc) as tc:
        with tc.tile_pool(name="persist", bufs=1) as pp:
            # resident activations
            qt_s = pp.tile([P, DC, NQB * QW], BF, tag="qt")   # QT [a, q]
            kt_s = pp.tile([P, DC, S], BF, tag="kt")          # KT [a, k]
            v_s = pp.tile([P, KT_TOT, D], BF, tag="v")        # V  [k, a]
            wo_s = pp.tile([P, DC, D], BF, tag="wo")          # Wo^T [a, m]
            bo_s = pp.tile([P, DC, 1], F32, tag="bo")
            ones_c = pp.tile([P, 1], F32, tag="ones_c")       # reduce lhsT
            ones_r = pp.tile([1, P], mybir.dt.float32r, tag="ones_r")  # rank-1 bcast lhsT
            ones_rf = pp.tile([1, P], F32, tag="ones_rf")
            nc.vector.memset(ones_c[:], 1.0)
            nc.vector.memset(ones_rf[:], 1.0)
            with nc.allow_low_precision(reason="1.0 is exact in fp22"):
                nc.vector.tensor_copy(ones_r[:], ones_rf[:])

            with tc.tile_pool(name="xw", bufs=1) as xw:
                # DMA issue order = consumption order: QT operands first so
                # the PE starts ~2us in, then KT's, then V's, then epilogue.
                xts, xtqs, wq_c, wk_c, wv_c = [], [], [], [], []
                for c in range(DC):
                    t = xw.tile([P, NQB * QW], BF, tag=f"xq{c}")
                    nc.sync.dma_start(t[:], xtq[c * P:(c + 1) * P, :])
                    xtqs.append(t)
                    t = xw.tile([P, D], BF, tag=f"wq{c}")
                    nc.sync.dma_start(t[:], wqt[c * P:(c + 1) * P, :])
                    wq_c.append(t)
                for c in range(DC):
                    t = xw.tile([P, S], BF, tag=f"xt{c}")
                    nc.sync.dma_start(t[:], xt[c * P:(c + 1) * P, :])
                    xts.append(t)
                for c in range(DC):
                    t = xw.tile([P, D], BF, tag=f"wk{c}")
                    nc.sync.dma_start(t[:], wkt[c * P:(c + 1) * P, :])
                    wk_c.append(t)
                for c in range(DC):
                    t = xw.tile([P, D], BF, tag=f"wv{c}")
                    nc.sync.dma_start(t[:], wvt[c * P:(c + 1) * P, :])
                    wv_c.append(t)
                nc.sync.dma_start(wo_s[:], wot.rearrange("(c p) m -> p c m", p=P))
                nc.sync.dma_start(bo_s[:], bo_d.rearrange("c p o -> p c o"))

                ps = tc.alloc_tile_pool(name="proj_psum", bufs=1, space="PSUM")

                # ---- projections (contraction over d, 8 chunks) ----
                # QT[a,q]: lhsT = wqT[d, a-tile], rhs = xtq[d, q-block]
                for at in range(DC):
                    for qb in range(NQB):
                        acc = ps.tile([P, QW], F32, tag="pp", bufs=2)
                        for d in range(DC):
                            nc.tensor.matmul(
                                acc[:],
                                wq_c[d][:, at * P:(at + 1) * P],
                                xtqs[d][:, qb * QW:(qb + 1) * QW],
                                start=(d == 0), stop=(d == DC - 1),
                            )
                        nc.vector.tensor_copy(
                            qt_s[:, at, qb * QW:(qb + 1) * QW], acc[:]
                        )
                # KT[a,k]: rhs = xt[d, k-block] (4 blocks of 512)
                for at in range(DC):
                    for kb in range(S // QW):
                        acc = ps.tile([P, QW], F32, tag="pp", bufs=2)
                        for d in range(DC):
                            nc.tensor.matmul(
                                acc[:],
                                wk_c[d][:, at * P:(at + 1) * P],
                                xts[d][:, kb * QW:(kb + 1) * QW],
                                start=(d == 0), stop=(d == DC - 1),
                            )
                        nc.vector.tensor_copy(
                            kt_s[:, at, kb * QW:(kb + 1) * QW], acc[:]
                        )
                # V[k,a]: lhsT = xt[d, k-tile], rhs = wvT[d, a-block]
                for kt in range(KT_TOT):
                    for ab in range(D // QW):
                        acc = ps.tile([P, QW], F32, tag="pp", bufs=2)
                        for d in range(DC):
                            nc.tensor.matmul(
                                acc[:],
                                xts[d][:, kt * P:(kt + 1) * P],
                                wv_c[d][:, ab * QW:(ab + 1) * QW],
                                start=(d == 0), stop=(d == DC - 1),
                            )
                        nc.vector.tensor_copy(
                            v_s[:, kt, ab * QW:(ab + 1) * QW], acc[:]
                        )
                ps.release()

            # ---- attention (per q-block) ----
            with (
                tc.tile_pool(name="attn", bufs=1) as ap,
                tc.tile_pool(name="attn_psum", bufs=1, space="PSUM") as ps,
            ):
                mask_s = ap.tile([P, KT_TOT, QW], BF, tag="mask")
                nc.sync.dma_start(mask_s[:], masks.rearrange("t p j -> p t j"))

                for qb in range(NQB):
                    nkt = NKT[qb]
                    kts = list(range(nkt))  # k-tiles used by this q-block
                    q_sl = slice(qb * QW, (qb + 1) * QW)

                    # ST[k,q] = KT.T @ QT, exp, mask -> PT (bf16, SBUF).
                    # The softmax denominator accumulates on DVE ([128,q]
                    # partial sums) so the PE only pays one reduce matmul.
                    pts = []
                    acc = ap.tile([P, QW], F32, tag="acc", bufs=2)
                    for i, kt in enumerate(kts):
                        st = ps.tile([P, QW], F32, tag="st", bufs=2)
                        for ac in range(DC):
                            nc.tensor.matmul(
                                st[:],
                                kt_s[:, ac, kt * P:(kt + 1) * P],
                                qt_s[:, ac, q_sl],
                                start=(ac == 0), stop=(ac == DC - 1),
                            )
                        pt = ap.tile([P, QW], BF, tag=f"pt{qb}_{kt}")
                        nc.scalar.activation(
                            pt[:], st[:], mybir.ActivationFunctionType.Exp,
                            scale=SCALE,
                        )
                        # qb0: every k-tile is masked; qb1: only tiles 8..15
                        if qb == 0 or kt >= 8:
                            nc.vector.tensor_mul(pt[:], pt[:], mask_s[:, kt, :])
                        if i == 0:
                            nc.vector.tensor_copy(acc[:], pt[:])
                        else:
                            nc.vector.tensor_add(acc[:], acc[:], pt[:])
                        pts.append(pt)

                    # Denominator: colsum[1,q] = ones.T @ acc (partition
                    # reduce), reciprocal, then rank-1 PE broadcast to
                    # [128,q] + ACT copy PSUM->SBUF. ~1us PE wait on the
                    # last exp/mask/add, then AV runs uninterrupted.
                    cs = ps.tile([1, QW], F32, tag="cs", bufs=1)
                    nc.tensor.matmul(cs[:], ones_c[:], acc[:], start=True, stop=True)
                    recip = ap.tile([1, QW], mybir.dt.float32r, tag="recip", bufs=2)
                    with nc.allow_low_precision(
                        reason="f32r (fp22) reciprocal row: 6e-5 rel err, "
                        "below this kernel's bf16 noise floor"
                    ):
                        nc.vector.reciprocal(recip[:], cs[:])
                    rb_ps = ps.tile([P, QW], F32, tag="rb", bufs=1)
                    nc.tensor.matmul(rb_ps[:], ones_r[:], recip[:], start=True, stop=True)
                    rb = ap.tile([P, QW], F32, tag="rb_sb", bufs=2)
                    nc.scalar.activation(
                        rb[:], rb_ps[:], mybir.ActivationFunctionType.Copy
                    )

                    # OT[a,q] = V.T @ PT (accumulate over k), normalize on copy
                    ots = []
                    for at in range(DC):
                        ot = ps.tile([P, QW], F32, tag="ot", bufs=2)
                        for i, kt in enumerate(kts):
                            nc.tensor.matmul(
                                ot[:],
                                v_s[:, kt, at * P:(at + 1) * P],
                                pts[i][:],
                                start=(i == 0), stop=(i == len(kts) - 1),
                            )
                        ot_sb = ap.tile([P, QW], BF, tag=f"ot{at}", bufs=2)
                        nc.vector.tensor_mul(ot_sb[:], ot[:], rb[:])
                        ots.append(ot_sb)

                    # YT[m,q] = WoT.T @ OT (accumulate over a) + bo
                    for mt in range(DC):
                        yt = ps.tile([P, QW], F32, tag="yt", bufs=2)
                        for ac in range(DC):
                            nc.tensor.matmul(
                                yt[:],
                                wo_s[:, ac, mt * P:(mt + 1) * P],
                                ots[ac][:],
                                start=(ac == 0), stop=(ac == DC - 1),
                            )
                        yt_sb = ap.tile([P, QW], BF, tag="yt_sb", bufs=2)
                        nc.vector.tensor_scalar_add(yt_sb[:], yt[:], bo_s[:, mt, :])
                        nc.sync.dma_start(
                            out[qb, mt * P:(mt + 1) * P, :], yt_sb[:]
                        )

    nc.compile()
    return nc


def _get_nc():
    global _NC_CACHE
    if _NC_CACHE is None:
        _NC_CACHE = _build()
    return _NC_CACHE


def kernel(x, mask, Wq, Wk, Wv, Wo, bo):
    global LAST_RESULT
    x = np.asarray(x, dtype=np.float32)
    mask = np.asarray(mask, dtype=np.float32)
    Wq = np.asarray(Wq, dtype=np.float32)
    Wk = np.asarray(Wk, dtype=np.float32)
    Wv = np.asarray(Wv, dtype=np.float32)
    Wo = np.asarray(Wo, dtype=np.float32)
    bo = np.asarray(bo, dtype=np.float32)

    wqt = Wq.T.astype(BF_NP).copy()
    wkt = Wk.T.astype(BF_NP).copy()
    wvt = Wv.T.astype(BF_NP).copy()
    wot = Wo.T.astype(BF_NP).copy()
    bo_r = np.ascontiguousarray(bo.reshape(DC, P, 1))

    in_maps = []
    for c in range(8):
        b, g = divmod(c, 2)
        xt = x[b].T.astype(BF_NP).copy()                       # [d, s]
        qcols = np.r_[QSTARTS[g][0]:QSTARTS[g][0] + QW,
                      QSTARTS[g][1]:QSTARTS[g][1] + QW]
        xtq = np.ascontiguousarray(xt[:, qcols])               # [d, 1024]

        m = np.zeros((KT_TOT, P, QW), dtype=np.float32)
        ki = np.arange(P)[:, None]
        qi = np.arange(QW)[None, :]
        for qb in range(NQB):
            q0 = QSTARTS[g][qb]
            for slot in range(8):
                kt = slot if qb == 0 else 8 + slot
                k0 = kt * P
                mm = ((k0 + ki) <= (q0 + qi)).astype(np.float32)
                mm *= mask[b, k0:k0 + P, None]                 # key padding
                m[kt] = mm
        in_maps.append({
            "xt": xt,
            "xtq": xtq,
            "wqt": wqt,
            "wkt": wkt,
            "wvt": wvt,
            "wot": wot,
            "bo": bo_r,
            "masks": m.astype(BF_NP),
        })

    nc = _get_nc()
    res = run_bass_kernel_spmd(
        nc, in_maps, core_ids=list(range(8)),
        trace=bool(os.environ.get("ATTN_TRACE")),
    )
    LAST_RESULT = res

    outp = np.empty((B, S, D), dtype=np.float32)
    for c in range(8):
        b, g = divmod(c, 2)
        yt = res.results[c]["out"]                     # [qb, m, q] bf16
        for qb in range(NQB):
            q0 = QSTARTS[g][qb]
            outp[b, q0:q0 + QW, :] = yt[qb].T.astype(np.float32)
    return outp
